# revision 5
# baseline (speedup 1.0000x reference)
"""Multi-head attention Trainium2 kernel (8 NeuronCores, SPMD).

Problem: B=4, S=2048, D_MODEL=1024, H=16, DIM=64 (nn_MultiHeadAttn).
Sharding: core c handles (batch b = c//2, query-row chunk c%2 of 1024).
Each core computes all 16 heads for its 1024 query rows against the full
2048 keys of its batch, then its rows of the output projection.

Device dataflow (host pre-transposes everything; device never transposes):
  - q^T/k^T/v^T arrive as [d_model, seq] bf16 so the PE contracts over
    the partition dim directly.
  - Per-head q/k projections via block-diagonal [128,128] weights: one
    matmul projects a pair of heads.  v is projected into natural [s, e]
    layout with an extra all-ones column appended per head.
  - scores^T[k,q] = (kh^T chunk).T @ qh^T; exp on ScalarE with the 1/8
    scale folded into the activation (no max subtraction needed --
    |scores| < ~3 so exp is safe and softmax is shift-invariant).
  - out_h^T[e,q] (+ sum of exp in row 64) = (vh ones-augmented).T @ attn^T
    accumulated over key chunks in PSUM.
  - normalize via fast reciprocal + partition-broadcast + multiply.
  - out^T[o,q] = Wo^T-tiles.T @ hidden^T accumulated over e-tiles.
"""

import sys

if "/opt/trn_rl_repo" not in sys.path:
    sys.path.insert(0, "/opt/trn_rl_repo")

import numpy as np
from contextlib import ExitStack

N_CORES = 8
B, S, D = 4, 2048, 1024
H, DIM = 16, 64
SQ = 1024          # query rows per core
NPAIR = 8          # head pairs
NKC = S // 128     # key chunks of 128
VAW = 130          # width of augmented V-projection weights (2*(64+1))

_cache = {}


def _build_program():
    from concourse import bacc, mybir, tile

    f32 = mybir.dt.float32
    bf16 = mybir.dt.bfloat16
    Exp = mybir.ActivationFunctionType.Exp

    nc = bacc.Bacc("TRN2", target_bir_lowering=False, debug=False)

    qT = nc.dram_tensor("qT", [D, SQ], bf16, kind="ExternalInput")
    kT = nc.dram_tensor("kT", [D, S], bf16, kind="ExternalInput")
    vT = nc.dram_tensor("vT", [D, S], bf16, kind="ExternalInput")
    wq2 = nc.dram_tensor("wq2", [128, 128], bf16, kind="ExternalInput")
    wk2 = nc.dram_tensor("wk2", [128, 128], bf16, kind="ExternalInput")
    wva = nc.dram_tensor("wva", [128, VAW], bf16, kind="ExternalInput")
    bq2 = nc.dram_tensor("bq2", [128, 1], f32, kind="ExternalInput")
    bk2 = nc.dram_tensor("bk2", [128, 1], f32, kind="ExternalInput")
    bva = nc.dram_tensor("bva", [128, VAW], f32, kind="ExternalInput")
    woT = nc.dram_tensor("woT", [D, D], bf16, kind="ExternalInput")
    bod = nc.dram_tensor("bod", [D, 1], f32, kind="ExternalInput")
    outT = nc.dram_tensor("outT", [D, SQ], f32, kind="ExternalOutput")

    with tile.TileContext(nc) as tc:
        with ExitStack() as ctx:
            ep = ctx.enter_context
            consts = ep(tc.tile_pool(name="consts", bufs=1))
            raw = ep(tc.tile_pool(name="raw", bufs=2))
            projq = ep(tc.tile_pool(name="projq", bufs=2))
            projk = ep(tc.tile_pool(name="projk", bufs=2))
            projv = ep(tc.tile_pool(name="projv", bufs=2))
            attn_p = ep(tc.tile_pool(name="attn", bufs=4))
            norm_p = ep(tc.tile_pool(name="norm", bufs=2))
            hid_p = ep(tc.tile_pool(name="hid", bufs=1))
            outs_p = ep(tc.tile_pool(name="outs", bufs=2))
            sc_ps = ep(tc.tile_pool(name="scps", bufs=2, space="PSUM"))
            av_ps = ep(tc.tile_pool(name="avps", bufs=2, space="PSUM"))

            def mm512(out, lhsT, rhs, start=True, stop=True):
                n = out.shape[-1]
                assert rhs.shape[-1] == n
                for j in range(0, n, 512):
                    w = min(512, n - j)
                    nc.tensor.matmul(out[..., j:j + w], lhsT, rhs[..., j:j + w],
                                     start=start, stop=stop)

            # ---- constants ----
            woT_s = consts.tile([128, 8, D], bf16, tag="woT")
            nc.sync.dma_start(woT_s[:], woT.rearrange("(et p) o -> p et o", p=128))
            bo_s = consts.tile([128, 8], f32, tag="bo")
            nc.sync.dma_start(bo_s[:], bod.rearrange("(ot p) one -> p (ot one)", p=128))
            wq2_s = consts.tile([128, 128], bf16, tag="wq2")
            nc.sync.dma_start(wq2_s[:], wq2[:, :])
            wk2_s = consts.tile([128, 128], bf16, tag="wk2")
            nc.sync.dma_start(wk2_s[:], wk2[:, :])
            wva_s = consts.tile([128, VAW], bf16, tag="wva")
            nc.sync.dma_start(wva_s[:], wva[:, :])
            bq2_s = consts.tile([128, 1], f32, tag="bq2")
            nc.sync.dma_start(bq2_s[:], bq2[:, :])
            bk2_s = consts.tile([128, 1], f32, tag="bk2")
            nc.sync.dma_start(bk2_s[:], bk2[:, :])
            bva_s = consts.tile([128, VAW], f32, tag="bva")
            nc.sync.dma_start(bva_s[:], bva[:, :])

            hidden = hid_p.tile([128, 8, SQ], bf16, tag="hidden")

            for pair in range(NPAIR):
                rows = slice(pair * 128, (pair + 1) * 128)
                # ---- stream raw inputs (transposed layout) ----
                q2 = raw.tile([128, SQ], bf16, tag="q2")
                nc.sync.dma_start(q2[:], qT[rows, :])
                k2 = raw.tile([128, S], bf16, tag="k2")
                nc.sync.dma_start(k2[:], kT[rows, :])
                v2 = raw.tile([128, S], bf16, tag="v2")
                nc.sync.dma_start(v2[:], vT[rows, :])

                # ---- Q projection: qh^T[e2, q] ----
                qh = projq.tile([128, SQ], bf16, tag="qh")
                ps = sc_ps.tile([128, SQ], f32, tag="sc")
                mm512(ps[:], wq2_s[:], q2[:])
                nc.vector.tensor_scalar_add(qh[:], ps[:], bq2_s[:])

                # ---- K projection: kh^T[e2, k] ----
                kh = projk.tile([128, S], bf16, tag="kh")
                for half in range(2):
                    ps = sc_ps.tile([128, SQ], f32, tag="sc")
                    mm512(ps[:], wk2_s[:],
                          k2[:, half * 1024:(half + 1) * 1024])
                    nc.vector.tensor_scalar_add(
                        kh[:, half * 1024:(half + 1) * 1024], ps[:], bk2_s[:])

                # ---- V projection (natural layout, with ones columns) ----
                vha = projv.tile([128, NKC, VAW], bf16, tag="vha")
                for sc_i in range(NKC):
                    psv = sc_ps.tile([128, VAW], f32, tag="sc")
                    nc.tensor.matmul(
                        psv[:], v2[:, sc_i * 128:(sc_i + 1) * 128], wva_s[:],
                        start=True, stop=True)
                    nc.vector.tensor_tensor(vha[:, sc_i, :], psv[:], bva_s[:],
                                            op=mybir.AluOpType.add)

                # ---- attention over this head pair ----
                avA = av_ps.tile([65, SQ], f32, tag="av")
                avB = av_ps.tile([65, SQ], f32, tag="av")
                for kc in range(NKC):
                    ks = slice(kc * 128, (kc + 1) * 128)
                    scA = sc_ps.tile([128, SQ], f32, tag="sc")
                    scB = sc_ps.tile([128, SQ], f32, tag="sc")
                    mm512(scA[:], kh[0:64, ks], qh[0:64, :])
                    mm512(scB[:], kh[64:128, ks], qh[64:128, :])
                    atA = attn_p.tile([128, SQ], bf16, tag="attn")
                    nc.scalar.activation(atA[:], scA[:], Exp, scale=0.125)
                    atB = attn_p.tile([128, SQ], bf16, tag="attn")
                    nc.scalar.activation(atB[:], scB[:], Exp, scale=0.125)
                    first, last = kc == 0, kc == NKC - 1
                    mm512(avA[:], vha[:, kc, 0:65], atA[:],
                          start=first, stop=last)
                    mm512(avB[:], vha[:, kc, 65:130], atB[:],
                          start=first, stop=last)

                # ---- normalize: hidden^T[e, q] = av[e, q] * (1/av[64, q]) ----
                # DVE ops cannot shift partitions, so the sum row (psum
                # partition 64) goes via an aligned copy + SBUF->SBUF DMA to
                # partition 0; head B's product is staged at partitions 0:64
                # and DMA'd into hidden partitions 64:128.
                for half, av in ((0, avA), (1, avB)):
                    s65 = norm_p.tile([65, SQ], f32, tag="s65")
                    nc.vector.tensor_copy(s65[64:65, :], av[64:65, :])
                    sums = norm_p.tile([1, SQ], f32, tag="sums")
                    nc.sync.dma_start(sums[:], s65[64:65, :])
                    recip = norm_p.tile([1, SQ], f32, tag="recip")
                    nc.vector.reciprocal_approx_fast(recip[:], sums[:])
                    rb = norm_p.tile([64, SQ], f32, tag="rb")
                    nc.gpsimd.partition_broadcast(rb[:], recip[:])
                    if half == 0:
                        nc.vector.tensor_tensor(
                            hidden[0:64, pair, :],
                            av[0:64, :], rb[:], op=mybir.AluOpType.mult)
                    else:
                        stg = norm_p.tile([64, SQ], bf16, tag="stg")
                        nc.vector.tensor_tensor(
                            stg[:], av[0:64, :], rb[:], op=mybir.AluOpType.mult)
                        nc.sync.dma_start(hidden[64:128, pair, :], stg[:])

            # ---- output projection: out^T[o, q] ----
            for ot in range(8):
                pso = sc_ps.tile([128, SQ], f32, tag="sc")
                for et in range(8):
                    mm512(pso[:],
                          woT_s[:, et, ot * 128:(ot + 1) * 128],
                          hidden[:, et, :],
                          start=(et == 0), stop=(et == 7))
                o_s = outs_p.tile([128, SQ], f32, tag="outs")
                nc.vector.tensor_scalar_add(o_s[:], pso[:], bo_s[:, ot:ot + 1])
                nc.sync.dma_start(outT[ot * 128:(ot + 1) * 128, :], o_s[:])

    nc.compile()
    return nc


def _get_nc():
    if "nc" not in _cache:
        _cache["nc"] = _build_program()
    return _cache["nc"]


def _prep_consts(Wq, bq, Wk, bk, Wv, bv, Wo, bo):
    import ml_dtypes
    f = np.float32
    b16 = ml_dtypes.bfloat16

    def blockdiag2(W):
        out = np.zeros((128, 128), f)
        out[:64, :64] = W.T
        out[64:, 64:] = W.T
        return out.astype(b16)

    wva = np.zeros((128, VAW), f)
    wva[:64, 0:64] = Wv.T          # head A
    wva[64:, 65:129] = Wv.T        # head B
    bva_row = np.zeros((VAW,), f)
    bva_row[0:64] = bv
    bva_row[64] = 1.0
    bva_row[65:129] = bv
    bva_row[129] = 1.0
    return {
        "wq2": blockdiag2(Wq),
        "wk2": blockdiag2(Wk),
        "wva": wva.astype(b16),
        "bq2": np.tile(bq.astype(f), 2)[:, None].copy(),
        "bk2": np.tile(bk.astype(f), 2)[:, None].copy(),
        "bva": np.broadcast_to(bva_row, (128, VAW)).copy(),
        "woT": np.ascontiguousarray(Wo.T.astype(f)).astype(b16),
        "bod": bo.astype(f)[:, None].copy(),
    }


def kernel(q, k, v, Wq, bq, Wk, bk, Wv, bv, Wo, bo, _trace=False):
    import ml_dtypes
    b16 = ml_dtypes.bfloat16
    q = np.asarray(q, np.float32)
    k = np.asarray(k, np.float32)
    v = np.asarray(v, np.float32)
    consts = _prep_consts(
        np.asarray(Wq, np.float32), np.asarray(bq, np.float32),
        np.asarray(Wk, np.float32), np.asarray(bk, np.float32),
        np.asarray(Wv, np.float32), np.asarray(bv, np.float32),
        np.asarray(Wo, np.float32), np.asarray(bo, np.float32))

    in_maps = []
    for c in range(N_CORES):
        b, chunk = c // 2, c % 2
        m = dict(consts)
        m["qT"] = np.ascontiguousarray(
            q[b, chunk * SQ:(chunk + 1) * SQ, :].T).astype(b16)
        m["kT"] = np.ascontiguousarray(k[b].T).astype(b16)
        m["vT"] = np.ascontiguousarray(v[b].T).astype(b16)
        in_maps.append(m)

    nc = _get_nc()
    from concourse.bass_utils import run_bass_kernel_spmd
    res = run_bass_kernel_spmd(nc, in_maps, core_ids=list(range(N_CORES)),
                               trace=_trace)
    if _trace:
        kernel.last_results = res

    out = np.empty((B, S, D), np.float32)
    for c in range(N_CORES):
        b, chunk = c // 2, c % 2
        out[b, chunk * SQ:(chunk + 1) * SQ, :] = res.results[c]["outT"].T
    return out
